# revision 8
# baseline (speedup 1.0000x reference)
"""GCN message-passing kernel for Trainium2, 8-core SPMD.

Strategy: shard dst nodes (and their incident edges) across 8 cores;
replicate small weights.  Per GCN layer, each core:
  1. computes its slice of table = dinv * (h @ W)   (separable gcn_norm)
  2. AllGather -> full [50000, 64] table in HBM
  3. dma_gather's its edges' src rows (the memory-bound part)
  4. scatter-adds via PE matmuls with host-built 0/1 count matrices
  5. epilogue: h' = relu(dinv * (agg + table_row) + b)
Final head: pooling via linearity (sum first, then Wn), tiny matmuls.
"""
import os
import sys

sys.path.insert(0, '/opt/trn_rl_repo')

import numpy as np

import concourse.bass as bass
import concourse.mybir as mybir
import concourse.tile as tile
from concourse import bacc, library_config
from concourse.bass_utils import run_bass_kernel_spmd

# ---------------- problem constants (hardcoded per spec) ----------------
N = 50000
E = 800000
CIN = 128
CH = 64
NL = int(os.environ.get("KERNEL_NL", "9"))
NCORES = 8
NLOC = N // NCORES            # 6250 dst nodes per core
NGR = 10                      # graphs
GN = 5000                     # nodes per graph
WIN = 32                      # dst window width (psum col-tile)
NWIN = (NLOC + WIN - 1) // WIN          # 196 (last window width 10)
NBLK = (NLOC + 127) // 128              # 49 dst blocks (last has 106 rows)
SUPB = 4                                # blocks per super
NSUP = (NBLK + SUPB - 1) // SUPB        # 13 supers
HALF = N // 2                           # table row split for int16 idx
GCHUNK = int(os.environ.get("KERNEL_GCHUNK", "896"))  # idx per dma_gather call (%128==0)

f32 = mybir.dt.float32
fp8 = mybir.dt.float8e4
bf16 = mybir.dt.bfloat16
i16 = mybir.dt.int16

LAST_EXEC_NS = None
LAST_TRACE = None


def _r32(x):
    return (int(x) + 31) // 32 * 32


def _win_width(w):
    return min(WIN, NLOC - w * WIN)


def _blk_rows(t):
    return min(128, NLOC - t * 128)


def _sup_blocks(s):
    return list(range(s * SUPB, min((s + 1) * SUPB, NBLK)))


def _sup_windows(s):
    ws = []
    for b in _sup_blocks(s):
        ws += [4 * b + j for j in range(4) if (4 * b + j) < NWIN]
    return ws


def build_structure(src, dst):
    """Uniform (core-independent) slot/segment/piece layout + per-core data.

    Returns (meta, per_core) where meta drives program emission (identical
    for all cores) and per_core holds the idx/S tensors.
    """
    core = dst // NLOC
    dl = dst % NLOC
    w = dl // WIN
    h = src // HALF
    sl = (src % HALF).astype(np.int64)

    key = ((core.astype(np.int64) * NWIN + w) * 2 + h)
    counts = np.bincount(key, minlength=NCORES * NWIN * 2).reshape(NCORES, NWIN, 2)
    mx = counts.max(axis=0)
    quota = np.maximum((mx + 31) // 32 * 32, 32)  # [NWIN, 2]; >=32 so h0 always inits

    # --- slot layout: supers -> halves -> windows (segment per (w, half))
    seg_off = np.zeros((NWIN, 2), np.int64)
    supers = []
    cur = 0
    nsub = 0
    for s in range(NSUP):
        sup = {"halves": {}, "scol0": nsub * WIN, "blocks": []}
        wins = _sup_windows(s)
        for hh in (0, 1):
            off = cur
            for ww in wins:
                seg_off[ww, hh] = cur
                cur += int(quota[ww, hh])
            sup["halves"][hh] = (off, cur - off)
        # subsegments + pieces (uniform)
        blocks = {}
        for b in _sup_blocks(s):
            blocks[b] = {"t": b, "wins": []}
        for b in _sup_blocks(s):
            for j in range(4):
                ww = 4 * b + j
                if ww >= NWIN:
                    continue
                wrec = {"w": ww, "width": _win_width(ww), "mpos": 32 * j,
                        "pieces": []}
                for hh in (0, 1):
                    roff, _rq = sup["halves"][hh]
                    o0 = int(seg_off[ww, hh] - roff)   # region-relative
                    q = int(quota[ww, hh])
                    # split [o0, o0+q) at 128 boundaries -> subsegs
                    o = o0
                    while o < o0 + q:
                        ln = min(128 - (o % 128), o0 + q - o)
                        sub_id = nsub
                        nsub += 1
                        # K-piece decomposition within the subseg
                        ro = o % 128
                        rem = ln
                        pos = ro
                        while rem > 0:
                            if pos % 128 == 0:
                                K = min(128, rem)
                            elif pos % 128 == 32:
                                K = min(32, rem)
                            elif pos % 128 == 64:
                                K = min(64, rem)
                            else:  # 96
                                K = min(32, rem)
                            wrec["pieces"].append(dict(
                                h=hh, gcol=o // 128, k_lo=pos, K=K,
                                sub=sub_id))
                            pos += K
                            rem -= K
                        o += ln
                grp = {}
                for i, pc in enumerate(wrec["pieces"]):
                    grp.setdefault(pc["k_lo"], []).append(i)
                wrec["groups"] = list(grp.keys())
                for idxs in grp.values():
                    for i in idxs:
                        wrec["pieces"][i]["start"] = (i == idxs[0])
                        wrec["pieces"][i]["stop"] = (i == idxs[-1])
                blocks[b]["wins"].append(wrec)
        sup["blocks"] = [blocks[b] for b in _sup_blocks(s)]
        sup["ncols"] = nsub * WIN - sup["scol0"]
        supers.append(sup)
    TOT = cur
    assert TOT % 32 == 0
    meta = dict(supers=supers, TOT=TOT, TOT16=TOT // 16, NSUB=nsub)

    # --- per-core content
    # slot index for every edge: seg_off[w,h] + rank within (core,w,h) group
    order = np.lexsort((h, w, core))
    ranks = np.empty(E, np.int64)
    ksorted = key[order]
    grp_start = np.r_[0, np.nonzero(np.diff(ksorted))[0] + 1]
    cc = np.arange(E, dtype=np.int64)
    rr = cc - np.repeat(grp_start, np.diff(np.r_[grp_start, E]))
    ranks[order] = rr
    slot = seg_off[w, h] + ranks   # global slot within the core's layout

    # subseg id for each edge (to locate its S column block)
    # recompute per-edge: region-relative offset & mapping (vectorized walk)
    # Build arrays indexed by (w, h): region_off, o0, first-piece-len, sub_base
    reg_off = np.zeros((NWIN, 2), np.int64)
    o0_arr = np.zeros((NWIN, 2), np.int64)
    subbase = np.zeros((NWIN, 2), np.int64)
    k = 0
    for s in range(NSUP):
        wins = _sup_windows(s)
        for hh in (0, 1):
            roff = supers[s]["halves"][hh][0]
            for ww in wins:
                reg_off[ww, hh] = roff
                o0_arr[ww, hh] = seg_off[ww, hh] - roff
    # subbase: walk again in the same order as above
    k = 0
    for s in range(NSUP):
        wins = _sup_windows(s)
        for b in _sup_blocks(s):
            pass
        for hh_b in ():
            pass
    # simpler: recompute subbase via identical loop
    k = 0
    for s in range(NSUP):
        for b in _sup_blocks(s):
            for j in range(4):
                ww = 4 * b + j
                if ww >= NWIN:
                    continue
                for hh in (0, 1):
                    subbase[ww, hh] = -1  # fill below
    k = 0
    for s in range(NSUP):
        for b in _sup_blocks(s):
            for j in range(4):
                ww = 4 * b + j
                if ww >= NWIN:
                    continue
                for hh in (0, 1):
                    o0 = int(o0_arr[ww, hh])
                    q = int(quota[ww, hh])
                    subbase[ww, hh] = k
                    # number of subsegs in this segment
                    o = o0
                    ns = 0
                    while o < o0 + q:
                        ln = min(128 - (o % 128), o0 + q - o)
                        ns += 1
                        o += ln
                    k += ns
    assert k == nsub

    per_core = []
    for c in range(NCORES):
        sel = core == c
        sl_c = sl[sel]
        w_c = w[sel]
        h_c = h[sel]
        dl_c = dl[sel]
        slot_c = slot[sel]
        # idx tensor
        idx_flat = np.zeros(TOT, np.int16)
        idx_flat[slot_c] = sl_c.astype(np.int16)
        idx_arr = np.tile(idx_flat.reshape(-1, 16).T, (8, 1)).astype(np.int16)
        # S tensor
        o = slot_c - reg_off[w_c, h_c]          # region-relative offset
        o0 = o0_arr[w_c, h_c]
        p0 = 128 - (o0 % 128)                   # first subseg length cap
        rel = o - o0
        subidx = np.where(rel < p0, 0, 1 + (rel - p0) // 128)
        sub = subbase[w_c, h_c] + subidx
        row = o % 128
        dcol = dl_c - w_c * WIN
        scol = sub * WIN + dcol
        S = np.zeros((128, nsub * WIN), np.float32)
        np.add.at(S, (row, scol), 1.0)
        per_core.append(dict(idx=idx_arr, S=S))
    return meta, per_core


def _host_tensors(x, edge_index, W0, b0, Ws, bs, Wn, Wp, Wa):
    src = edge_index[0].astype(np.int64)
    dst = edge_index[1].astype(np.int64)
    deg = np.bincount(dst, minlength=N).astype(np.float64) + 1.0
    dinv = (1.0 / np.sqrt(deg)).astype(np.float32)

    meta, per_core = build_structure(src, dst)

    b_all = np.concatenate([b0[None, :], bs], axis=0).astype(np.float32)[:NL]
    Ws = Ws[:max(0, NL - 1)]
    b_exp = np.tile(b_all.reshape(1, NL * CH), (128, 1)).astype(np.float32)
    ident = np.eye(128, dtype=np.float32)

    ins = []
    for c in range(NCORES):
        lo = c * NLOC
        xs = np.ascontiguousarray(x[lo:lo + NLOC]).astype(np.float32)
        dv = np.zeros((128, NBLK), np.float32)
        dvl = dinv[lo:lo + NLOC]
        for t in range(NBLK):
            r = _blk_rows(t)
            dv[:r, t] = dvl[t * 128:t * 128 + r]
        # head masks
        gA = lo // GN
        gB = gA + 1
        pm = np.zeros((128, NBLK, 2), np.float32)
        mA = np.zeros((128, NBLK), np.float32)
        mB = np.zeros((128, NBLK), np.float32)
        node = lo + np.arange(NLOC)
        gof = node // GN
        for t in range(NBLK):
            r = _blk_rows(t)
            g_t = gof[t * 128:t * 128 + r]
            pm[:r, t, 0] = (g_t == gA)
            pm[:r, t, 1] = (g_t == gB)
            mA[:r, t] = (g_t == gA)
            mB[:r, t] = (g_t == gB)
        ohp = np.zeros((2, 16), np.float32)
        ohp[0, gA] = 1.0
        if gB < NGR:
            ohp[1, gB] = 1.0
        oh2 = np.zeros((16, 256), np.float32)
        oh2[gA, 0:128] = 1.0
        if gB < NGR:
            oh2[gB, 128:256] = 1.0
        ins.append({
            "x_sl": xs,
            "idx_in": per_core[c]["idx"],
            "s_in": per_core[c]["S"].astype(mybir.dt.np(bf16)),
            "dinv_t": dv,
            "W0_in": W0.astype(np.float32),
            "Ws_in": np.ascontiguousarray(
                Ws.transpose(1, 0, 2).reshape(CH, (NL - 1) * CH)
            ).astype(np.float32) if NL > 1 else np.zeros((CH, max(1, (NL - 1) * CH)), np.float32),
            "b_exp": b_exp,
            "ident_in": ident,
            "Wn_in": Wn.astype(np.float32),
            "Wp_in": Wp.astype(np.float32),
            "Wa1_in": Wa[:CH].astype(np.float32),
            "Wa2_in": Wa[CH:].astype(np.float32),
            "pm_in": pm.reshape(128, NBLK * 2),
            "mA_in": mA,
            "mB_in": mB,
            "ohp_in": ohp,
            "oh2_in": oh2,
        })
    return meta, ins


def build_nc(meta):
    AL = mybir.AluOpType
    AF = mybir.ActivationFunctionType
    from concourse.bass import broadcast_tensor_aps
    nc = bacc.Bacc("TRN2", num_devices=NCORES, target_bir_lowering=False,
                   debug=False, num_swdge_queues=4)

    def ts_mult(out_ap, in_ap, col_ap):
        # per-partition scale via stride-0 broadcast; avoids the DVE
        # const-pointer tensor_scalar path (~1.3us per op)
        a, b = broadcast_tensor_aps(in_ap, col_ap)
        nc.vector.tensor_tensor(out_ap, a, b, op=AL.mult)
    TOT16 = meta["TOT16"]
    NSUB = meta["NSUB"]
    supers = meta["supers"]

    # ---- dram I/O
    x_d = nc.dram_tensor("x_sl", [NLOC, CIN], f32, kind="ExternalInput")
    idx_d = nc.dram_tensor("idx_in", [128, TOT16], i16, kind="ExternalInput")
    s_d = nc.dram_tensor("s_in", [128, NSUB * WIN], bf16, kind="ExternalInput")
    dinv_d = nc.dram_tensor("dinv_t", [128, NBLK], f32, kind="ExternalInput")
    W0_d = nc.dram_tensor("W0_in", [CIN, CH], f32, kind="ExternalInput")
    Ws_d = nc.dram_tensor("Ws_in", [CH, max(1, (NL - 1) * CH)], f32,
                          kind="ExternalInput")
    b_d = nc.dram_tensor("b_exp", [128, NL * CH], f32, kind="ExternalInput")
    id_d = nc.dram_tensor("ident_in", [128, 128], f32, kind="ExternalInput")
    Wn_d = nc.dram_tensor("Wn_in", [CH, CH], f32, kind="ExternalInput")
    Wp_d = nc.dram_tensor("Wp_in", [CH, CH], f32, kind="ExternalInput")
    Wa1_d = nc.dram_tensor("Wa1_in", [CH, 1], f32, kind="ExternalInput")
    Wa2_d = nc.dram_tensor("Wa2_in", [CH, 1], f32, kind="ExternalInput")
    pm_d = nc.dram_tensor("pm_in", [128, NBLK * 2], f32, kind="ExternalInput")
    mA_d = nc.dram_tensor("mA_in", [128, NBLK], f32, kind="ExternalInput")
    mB_d = nc.dram_tensor("mB_in", [128, NBLK], f32, kind="ExternalInput")
    ohp_d = nc.dram_tensor("ohp_in", [2, 16], f32, kind="ExternalInput")
    oh2_d = nc.dram_tensor("oh2_in", [16, 256], f32, kind="ExternalInput")
    y_d = nc.dram_tensor("y", [NLOC, 1], f32, kind="ExternalOutput")

    with tile.TileContext(nc) as tc:
        import contextlib
        with contextlib.ExitStack() as ctx:
            cp = ctx.enter_context(tc.tile_pool(name="const", bufs=1))
            sa = ctx.enter_context(tc.tile_pool(name="stageA", bufs=3))
            hp = ctx.enter_context(tc.tile_pool(name="hbuf", bufs=2))
            tp = ctx.enter_context(tc.tile_pool(name="tbl", bufs=2))
            gp = ctx.enter_context(tc.tile_pool(name="gbuf", bufs=3))
            ep = ctx.enter_context(tc.tile_pool(name="epi", bufs=3))
            dp = ctx.enter_context(tc.tile_pool(name="dram", bufs=2,
                                                space="DRAM"))
            psA = ctx.enter_context(tc.tile_pool(name="psA", bufs=3,
                                                 space="PSUM"))
            psB = ctx.enter_context(tc.tile_pool(name="psB", bufs=5,
                                                 space="PSUM"))

            nc.gpsimd.load_library(library_config.mlp)

            def ld(tag, shape, dt_, dram):
                t = cp.tile(shape, dt_, tag=tag)
                nc.sync.dma_start(t[:], dram[:])
                return t

            idx_sb = ld("c_idx", [128, TOT16], i16, idx_d)
            s8_sb = ld("c_s8", [128, NSUB * WIN], bf16, s_d)
            dinv_sb = ld("c_dinv", [128, NBLK], f32, dinv_d)
            W0_sb = ld("c_W0", [CIN, CH], f32, W0_d)
            Ws_sb = ld("c_Ws", [CH, max(1, (NL - 1) * CH)], f32, Ws_d)
            b_sb = ld("c_b", [128, NL * CH], f32, b_d)
            id_sb = ld("c_id", [128, 128], f32, id_d)
            Wn_sb = ld("c_Wn", [CH, CH], f32, Wn_d)
            Wp_sb = ld("c_Wp", [CH, CH], f32, Wp_d)
            Wa1_sb = ld("c_Wa1", [CH, 1], f32, Wa1_d)
            Wa2_sb = ld("c_Wa2", [CH, 1], f32, Wa2_d)
            pm_sb = ld("c_pm", [128, NBLK * 2], f32, pm_d)
            mA_sb = ld("c_mA", [128, NBLK], f32, mA_d)
            mB_sb = ld("c_mB", [128, NBLK], f32, mB_d)
            ohp_sb = ld("c_ohp", [2, 16], f32, ohp_d)
            oh2_sb = ld("c_oh2", [16, 256], f32, oh2_d)

            def stage_a_block(in_ap, W_ap, Cin, tbl_t, r, t):
                p1 = psA.tile([128, 128], f32, tag="ps")
                nc.tensor.transpose(p1[:Cin, :r], in_ap, id_sb[:r, :r])
                hT = sa.tile([128, 128], f32, tag="hT")
                nc.scalar.copy(hT[:Cin, :r], p1[:Cin, :r])
                p2 = psA.tile([CH, 128], f32, tag="ps")
                nc.tensor.matmul(p2[:, :r], W_ap, hT[:Cin, :r],
                                 start=True, stop=True)
                hwT = sa.tile([CH, 128], f32, tag="hwT")
                nc.scalar.copy(hwT[:, :r], p2[:, :r])
                p3 = psA.tile([128, CH], f32, tag="ps")
                nc.tensor.transpose(p3[:r, :], hwT[:, :r], id_sb[:CH, :CH])
                ts_mult(tbl_t[:r, t, 0:CH], p3[:r, :], dinv_sb[:r, t:t + 1])

            def do_allgather(tbl_t):
                bounce = dp.tile([NLOC, 2 * CH], bf16, tag="bounce")
                nfull = (NBLK - 1) * 128
                nc.sync.dma_start(
                    bounce[0:nfull, :].rearrange("(t p) c -> p t c", p=128),
                    tbl_t[:, 0:NBLK - 1, :])
                lr = _blk_rows(NBLK - 1)
                nc.sync.dma_start(bounce[nfull:nfull + lr, :],
                                  tbl_t[0:lr, NBLK - 1, :])
                table_t = dp.tile([N, 2 * CH], bf16, tag="table",
                                  addr_space="Shared")
                nc.gpsimd.collective_compute(
                    "AllGather", AL.bypass,
                    replica_groups=[list(range(NCORES))],
                    ins=[bounce.opt()], outs=[table_t.opt()])
                return table_t

            s1_sb = cp.tile([128, NBLK], f32, tag="s1")

            def s1_block(h_t, r, t):
                q1 = psA.tile([CH, 128], f32, tag="ps")
                nc.tensor.transpose(q1[:, :r], h_t[:r, t, :], id_sb[:r, :r])
                hT9 = sa.tile([CH, 128], f32, tag="hT9")
                nc.scalar.copy(hT9[:, :r], q1[:, :r])
                q2 = psA.tile([CH, 128], f32, tag="ps")
                nc.tensor.matmul(q2[:, :r], Wn_sb[:], hT9[:, :r], start=True,
                                 stop=True)
                rl = sa.tile([CH, 128], f32, tag="rl")
                nc.scalar.activation(rl[:, :r], q2[:, :r], AF.Relu)
                q3 = psA.tile([128, 1], f32, tag="ps")
                nc.tensor.matmul(q3[:r, :], rl[:, :r], Wa1_sb[:], start=True,
                                 stop=True, tile_position=(0, 0))
                nc.vector.tensor_copy(s1_sb[:r, t:t + 1], q3[:r, :])

            # ------- prologue: stage A for layer 0 (from x) + AllGather ----
            tbl = tp.tile([128, NBLK, 2 * CH], bf16, tag="tbl")
            for t in range(NBLK):
                r = _blk_rows(t)
                xin = sa.tile([128, CIN], f32, tag="xin")
                nc.sync.dma_start(xin[:r, :], x_d[t * 128:t * 128 + r, :])
                stage_a_block(xin[:r, :], W0_sb[:CIN, :], CIN, tbl, r, t)
            table = do_allgather(tbl)

            h_cur = None
            for layer in range(NL):
                # stage A of layer+1 is pipelined into this layer's scatter;
                # its table is all-gathered at the end of this iteration.
                if layer < NL - 1:
                    tbl_nx = tp.tile([128, NBLK, 2 * CH], bf16, tag="tbl",
                                     name=f"tblnx{layer}")
                    W_nx = Ws_sb[:CH, layer * CH:(layer + 1) * CH]
                else:
                    tbl_nx = None
                    W_nx = None
                # ------- stage B: gather + scatter matmuls -------
                h_next = hp.tile([128, NBLK, CH], f32, tag="h")
                DBG = int(os.environ.get("KERNEL_DEBUG", "0"))
                _gq = [0]
                for s, sup in enumerate(supers):
                    G = {}
                    for hh in (0, 1):
                        off, q = sup["halves"][hh]
                        g = gp.tile([128, (q + 127) // 128, 2 * CH], bf16,
                                    tag="G")
                        if DBG >= 2:
                            nc.vector.memset(g[:], 0.0)
                        else:
                            a = 0
                            while a < q:
                                nn_ = min(GCHUNK, q - a)
                                nc.gpsimd.dma_gather(
                                    g[:, a // 128:(a + nn_ + 127) // 128, :],
                                    table[hh * HALF:(hh + 1) * HALF, :],
                                    idx_sb[:, (off + a) // 16:
                                           (off + a + nn_) // 16],
                                    nn_, nn_, 2 * CH,
                                    queue_num=_gq[0] % 4)
                                _gq[0] += 1
                                a += nn_
                        G[hh] = g

                    for brec in sup["blocks"]:
                        t = brec["t"]
                        r = _blk_rows(t)
                        agg = ep.tile([128, CH], f32, tag="agg")
                        for wrec in brec["wins"]:
                            width = wrec["width"]
                            mp = wrec["mpos"]
                            pieces = (wrec["pieces"] if DBG < 1
                                      else wrec["pieces"][:1])
                            for pc in pieces:
                                st = pc["start"] if DBG < 1 else True
                                sp_ = pc["stop"] if DBG < 1 else True
                                pw = brec.setdefault("_pw", {})
                                if wrec["w"] not in pw:
                                    pwt_new = psB.tile(
                                        [128, CH], f32, tag="pt", name=f"pw{wrec['w']}")
                                    pw[wrec["w"]] = pwt_new
                                scol_abs = pc["sub"] * WIN
                                nc.tensor.matmul(
                                    pw[wrec["w"]][pc["k_lo"]:
                                                  pc["k_lo"] + width, :],
                                    s8_sb[pc["k_lo"]:pc["k_lo"] + pc["K"],
                                          scol_abs:scol_abs + width],
                                    G[pc["h"]][pc["k_lo"]:pc["k_lo"] + pc["K"],
                                               pc["gcol"], 0:CH],
                                    start=st, stop=sp_,
                                    tile_position=(pc["k_lo"], pc["k_lo"]))
                            # fold the row-group slots into agg[mp:mp+width]
                            pwt = brec["_pw"][wrec["w"]]
                            gs = wrec["groups"] if DBG < 1 else wrec["groups"][:1]
                            nc.vector.tensor_copy(
                                agg[mp:mp + width, :],
                                pwt[gs[0]:gs[0] + width, :])
                            for gk in gs[1:]:
                                nc.vector.tensor_tensor(
                                    agg[mp:mp + width, :],
                                    agg[mp:mp + width, :],
                                    pwt[gk:gk + width, :], op=AL.add)
                        brec.pop("_pw", None)
                        # epilogue: h' = relu(dinv*(agg + table_row) + b)
                        tmp = ep.tile([128, CH], f32, tag="tmp")
                        nc.vector.tensor_tensor(tmp[:r, :], agg[:r, :],
                                                tbl[:r, t, 0:CH], op=AL.add)
                        ts_mult(tmp[:r, :], tmp[:r, :], dinv_sb[:r, t:t + 1])
                        nc.vector.tensor_tensor(
                            tmp[:r, :], tmp[:r, :],
                            b_sb[:r, layer * CH:(layer + 1) * CH], op=AL.add)
                        nc.scalar.activation(h_next[:r, t, :], tmp[:r, :],
                                             AF.Relu)
                        # pipelined: next layer's table row / head s1 term
                        if tbl_nx is not None:
                            stage_a_block(h_next[:r, t, :], W_nx, CH,
                                          tbl_nx, r, t)
                        elif layer == NL - 1:
                            s1_block(h_next, r, t)
                if tbl_nx is not None:
                    tbl = tbl_nx
                    table = do_allgather(tbl)
                h_cur = h_next

            # ---------------- head ----------------
            # pool partials [2, 64] via mask matmuls
            pp = psB.tile([2, CH], f32, tag="pt")
            for t in range(NBLK):
                r = _blk_rows(t)
                nc.tensor.matmul(pp[:, :], pm_sb[:r, 2 * t:2 * t + 2],
                                 h_cur[:r, t, :],
                                 start=(t == 0), stop=(t == NBLK - 1),
                                 tile_position=(0, 0))
            pp_sb = sa.tile([2, CH], f32, tag="pp")
            nc.vector.tensor_copy(pp_sb[:], pp[:])
            pg = psB.tile([16, CH], f32, tag="pt")
            nc.tensor.matmul(pg[:], ohp_sb[:], pp_sb[:], start=True, stop=True,
                             tile_position=(0, 0))
            pg_sb = sa.tile([16, CH], f32, tag="pg")
            nc.vector.tensor_copy(pg_sb[:], pg[:])
            b2 = dp.tile([16, CH], f32, tag="b2")
            nc.sync.dma_start(b2[:], pg_sb[:])
            ar = dp.tile([16, CH], f32, tag="ar", addr_space="Shared")
            nc.gpsimd.collective_compute(
                "AllReduce", AL.add, replica_groups=[list(range(NCORES))],
                ins=[b2.opt()], outs=[ar.opt()])
            pool_sb = sa.tile([16, CH], f32, tag="pool")
            nc.sync.dma_start(pool_sb[:], ar[:])
            # poolT [64, 16]
            pT = psA.tile([CH, 16], f32, tag="ps")
            nc.tensor.transpose(pT[:], pool_sb[:], id_sb[:16, :16])
            pT_sb = sa.tile([CH, 16], f32, tag="pT")
            nc.vector.tensor_copy(pT_sb[:], pT[:])
            pWn = psA.tile([CH, 16], f32, tag="ps")
            nc.tensor.matmul(pWn[:], Wn_sb[:], pT_sb[:], start=True, stop=True)
            pWn_sb = sa.tile([CH, 16], f32, tag="pWn")
            nc.vector.tensor_copy(pWn_sb[:], pWn[:])
            rep = psA.tile([CH, 16], f32, tag="ps")
            nc.tensor.matmul(rep[:], Wp_sb[:], pWn_sb[:], start=True, stop=True)
            repr_sb = sa.tile([CH, 16], f32, tag="repr")
            nc.scalar.activation(repr_sb[:], rep[:], AF.Relu)
            s2 = psA.tile([16, 1], f32, tag="ps")
            nc.tensor.matmul(s2[:], repr_sb[:], Wa2_sb[:], start=True,
                             stop=True, tile_position=(0, 0))
            s2_sb = sa.tile([16, 1], f32, tag="s2")
            nc.vector.tensor_copy(s2_sb[:], s2[:])
            s2A = psA.tile([128, 1], f32, tag="ps")
            nc.tensor.matmul(s2A[:], oh2_sb[:, 0:128], s2_sb[:], start=True,
                             stop=True, tile_position=(0, 0))
            s2A_sb = sa.tile([128, 1], f32, tag="s2A")
            nc.vector.tensor_copy(s2A_sb[:], s2A[:])
            s2B = psA.tile([128, 1], f32, tag="ps")
            nc.tensor.matmul(s2B[:], oh2_sb[:, 128:256], s2_sb[:], start=True,
                             stop=True, tile_position=(0, 0))
            s2B_sb = sa.tile([128, 1], f32, tag="s2B")
            nc.vector.tensor_copy(s2B_sb[:], s2B[:])
            # s1 per tile was computed inline during the last layer's scatter
            # s2e + tanh
            e1 = sa.tile([128, NBLK], f32, tag="e1")
            ts_mult(e1[:], mA_sb[:], s2A_sb[:, 0:1])
            e2 = sa.tile([128, NBLK], f32, tag="e2")
            ts_mult(e2[:], mB_sb[:], s2B_sb[:, 0:1])
            nc.vector.tensor_tensor(e1[:], e1[:], e2[:], op=AL.add)
            nc.vector.tensor_tensor(e1[:], e1[:], s1_sb[:], op=AL.add)
            yo = cp.tile([128, NBLK, 1], f32, tag="yo")
            _fn = (AF.Copy if os.environ.get("KERNEL_NOTANH", "0") == "1"
                   else AF.Tanh)
            nc.scalar.activation(yo[:, :, 0], e1[:], _fn)
            nfull = (NBLK - 1) * 128
            nc.sync.dma_start(
                y_d[0:nfull, :].rearrange("(t p) c -> p t c", p=128),
                yo[:, 0:NBLK - 1, :])
            lr = _blk_rows(NBLK - 1)
            nc.sync.dma_start(y_d[nfull:nfull + lr, :], yo[0:lr, NBLK - 1, :])
    nc.compile()
    return nc


def kernel(x, edge_index, W0, b0, Ws, bs, Wn, Wp, Wa, n):
    global LAST_EXEC_NS, LAST_TRACE
    assert int(n) == GN
    x = np.asarray(x, np.float32)
    edge_index = np.asarray(edge_index)
    meta, ins = _host_tensors(np.asarray(x), np.asarray(edge_index),
                              np.asarray(W0), np.asarray(b0), np.asarray(Ws),
                              np.asarray(bs), np.asarray(Wn), np.asarray(Wp),
                              np.asarray(Wa))
    nc = build_nc(meta)
    trace = os.environ.get("KERNEL_TRACE", "0") == "1"
    if trace:
        import types
        try:
            import antenv.axon_hooks  # noqa: F401
        except ImportError:
            from trn_agent_boot.trn_boot import _ntff_profile_via_ctypes
            _hook = _ntff_profile_via_ctypes('/opt/axon/libaxon_pjrt.so')
            mod = types.ModuleType("antenv.axon_hooks")
            mod.get_axon_ntff_profile_hook = lambda: _hook
            sys.modules['antenv.axon_hooks'] = mod
    res = run_bass_kernel_spmd(
        nc, ins, core_ids=list(range(NCORES)), trace=trace,
        trace_cores=list(range(NCORES)) if trace else None)
    LAST_EXEC_NS = res.exec_time_ns
    LAST_TRACE = (res.instructions_and_trace[1]
                  if res.instructions_and_trace else None)
    out = np.empty((N, 1), np.float32)
    for c in range(NCORES):
        out[c * NLOC:(c + 1) * NLOC] = res.results[c]["y"]
    return out



# revision 11
# speedup vs baseline: 1.1307x; 1.1307x over previous
"""GCN message-passing kernel for Trainium2, 8-core SPMD.

Strategy: shard dst nodes (and their incident edges) across 8 cores;
replicate small weights.  Per GCN layer, each core:
  1. computes its slice of table = dinv * (h @ W)   (separable gcn_norm)
  2. AllGather -> full [50000, 64] table in HBM
  3. dma_gather's its edges' src rows (the memory-bound part)
  4. scatter-adds via PE matmuls with host-built 0/1 count matrices
  5. epilogue: h' = relu(dinv * (agg + table_row) + b)
Final head: pooling via linearity (sum first, then Wn), tiny matmuls.
"""
import os
import sys

sys.path.insert(0, '/opt/trn_rl_repo')

import numpy as np

import concourse.bass as bass
import concourse.mybir as mybir
import concourse.tile as tile
from concourse import bacc, library_config
from concourse.bass_utils import run_bass_kernel_spmd

# ---------------- problem constants (hardcoded per spec) ----------------
N = 50000
E = 800000
CIN = 128
CH = 64
NL = int(os.environ.get("KERNEL_NL", "9"))
NCORES = 8
NLOC = N // NCORES            # 6250 dst nodes per core
NGR = 10                      # graphs
GN = 5000                     # nodes per graph
WIN = 32                      # dst window width (psum col-tile)
NWIN = (NLOC + WIN - 1) // WIN          # 196 (last window width 10)
NBLK = (NLOC + 127) // 128              # 49 dst blocks (last has 106 rows)
SUPB = 4                                # blocks per super
NSUP = (NBLK + SUPB - 1) // SUPB        # 13 supers
HALF = N // 2                           # table row split for int16 idx
GCHUNK = int(os.environ.get("KERNEL_GCHUNK", "896"))  # idx per dma_gather call (%128==0)

f32 = mybir.dt.float32
fp8 = mybir.dt.float8e4
bf16 = mybir.dt.bfloat16
i16 = mybir.dt.int16

LAST_EXEC_NS = None
LAST_TRACE = None


def _r32(x):
    return (int(x) + 31) // 32 * 32


def _win_width(w):
    return min(WIN, NLOC - w * WIN)


def _blk_rows(t):
    return min(128, NLOC - t * 128)


def _sup_blocks(s):
    return list(range(s * SUPB, min((s + 1) * SUPB, NBLK)))


def _sup_windows(s):
    ws = []
    for b in _sup_blocks(s):
        ws += [4 * b + j for j in range(4) if (4 * b + j) < NWIN]
    return ws


def build_structure(src, dst):
    """Uniform (core-independent) slot/segment/piece layout + per-core data.

    Returns (meta, per_core) where meta drives program emission (identical
    for all cores) and per_core holds the idx/S tensors.
    """
    core = dst // NLOC
    dl = dst % NLOC
    w = dl // WIN
    h = src // HALF
    sl = (src % HALF).astype(np.int64)

    key = ((core.astype(np.int64) * NWIN + w) * 2 + h)
    counts = np.bincount(key, minlength=NCORES * NWIN * 2).reshape(NCORES, NWIN, 2)
    mx = counts.max(axis=0)
    quota = np.maximum((mx + 31) // 32 * 32, 32)  # [NWIN, 2]; >=32 so h0 always inits

    # --- slot layout: supers -> halves -> windows (segment per (w, half))
    seg_off = np.zeros((NWIN, 2), np.int64)
    supers = []
    cur = 0
    nsub = 0
    for s in range(NSUP):
        sup = {"halves": {}, "scol0": nsub * WIN, "blocks": []}
        wins = _sup_windows(s)
        for hh in (0, 1):
            off = cur
            for ww in wins:
                seg_off[ww, hh] = cur
                cur += int(quota[ww, hh])
            sup["halves"][hh] = (off, cur - off)
        # subsegments + pieces (uniform)
        blocks = {}
        for b in _sup_blocks(s):
            blocks[b] = {"t": b, "wins": []}
        for b in _sup_blocks(s):
            for j in range(4):
                ww = 4 * b + j
                if ww >= NWIN:
                    continue
                wrec = {"w": ww, "width": _win_width(ww), "mpos": 32 * j,
                        "pieces": []}
                for hh in (0, 1):
                    roff, _rq = sup["halves"][hh]
                    o0 = int(seg_off[ww, hh] - roff)   # region-relative
                    q = int(quota[ww, hh])
                    # split [o0, o0+q) at 128 boundaries -> subsegs
                    o = o0
                    while o < o0 + q:
                        ln = min(128 - (o % 128), o0 + q - o)
                        sub_id = nsub
                        nsub += 1
                        # K-piece decomposition within the subseg
                        ro = o % 128
                        rem = ln
                        pos = ro
                        while rem > 0:
                            if pos % 128 == 0:
                                K = min(128, rem)
                            elif pos % 128 == 32:
                                K = min(32, rem)
                            elif pos % 128 == 64:
                                K = min(64, rem)
                            else:  # 96
                                K = min(32, rem)
                            wrec["pieces"].append(dict(
                                h=hh, gcol=o // 128, k_lo=pos, K=K,
                                sub=sub_id))
                            pos += K
                            rem -= K
                        o += ln
                grp = {}
                for i, pc in enumerate(wrec["pieces"]):
                    grp.setdefault(pc["k_lo"], []).append(i)
                wrec["groups"] = list(grp.keys())
                for idxs in grp.values():
                    for i in idxs:
                        wrec["pieces"][i]["start"] = (i == idxs[0])
                        wrec["pieces"][i]["stop"] = (i == idxs[-1])
                blocks[b]["wins"].append(wrec)
        sup["blocks"] = [blocks[b] for b in _sup_blocks(s)]
        sup["ncols"] = nsub * WIN - sup["scol0"]
        supers.append(sup)
    TOT = cur
    assert TOT % 32 == 0
    meta = dict(supers=supers, TOT=TOT, TOT16=TOT // 16, NSUB=nsub)

    # --- per-core content
    # slot index for every edge: seg_off[w,h] + rank within (core,w,h) group
    order = np.lexsort((h, w, core))
    ranks = np.empty(E, np.int64)
    ksorted = key[order]
    grp_start = np.r_[0, np.nonzero(np.diff(ksorted))[0] + 1]
    cc = np.arange(E, dtype=np.int64)
    rr = cc - np.repeat(grp_start, np.diff(np.r_[grp_start, E]))
    ranks[order] = rr
    slot = seg_off[w, h] + ranks   # global slot within the core's layout

    # subseg id for each edge (to locate its S column block)
    # recompute per-edge: region-relative offset & mapping (vectorized walk)
    # Build arrays indexed by (w, h): region_off, o0, first-piece-len, sub_base
    reg_off = np.zeros((NWIN, 2), np.int64)
    o0_arr = np.zeros((NWIN, 2), np.int64)
    subbase = np.zeros((NWIN, 2), np.int64)
    k = 0
    for s in range(NSUP):
        wins = _sup_windows(s)
        for hh in (0, 1):
            roff = supers[s]["halves"][hh][0]
            for ww in wins:
                reg_off[ww, hh] = roff
                o0_arr[ww, hh] = seg_off[ww, hh] - roff
    # subbase: walk again in the same order as above
    k = 0
    for s in range(NSUP):
        wins = _sup_windows(s)
        for b in _sup_blocks(s):
            pass
        for hh_b in ():
            pass
    # simpler: recompute subbase via identical loop
    k = 0
    for s in range(NSUP):
        for b in _sup_blocks(s):
            for j in range(4):
                ww = 4 * b + j
                if ww >= NWIN:
                    continue
                for hh in (0, 1):
                    subbase[ww, hh] = -1  # fill below
    k = 0
    for s in range(NSUP):
        for b in _sup_blocks(s):
            for j in range(4):
                ww = 4 * b + j
                if ww >= NWIN:
                    continue
                for hh in (0, 1):
                    o0 = int(o0_arr[ww, hh])
                    q = int(quota[ww, hh])
                    subbase[ww, hh] = k
                    # number of subsegs in this segment
                    o = o0
                    ns = 0
                    while o < o0 + q:
                        ln = min(128 - (o % 128), o0 + q - o)
                        ns += 1
                        o += ln
                    k += ns
    assert k == nsub

    per_core = []
    for c in range(NCORES):
        sel = core == c
        sl_c = sl[sel]
        w_c = w[sel]
        h_c = h[sel]
        dl_c = dl[sel]
        slot_c = slot[sel]
        # idx tensor
        idx_flat = np.zeros(TOT, np.int16)
        idx_flat[slot_c] = sl_c.astype(np.int16)
        idx_arr = np.tile(idx_flat.reshape(-1, 16).T, (8, 1)).astype(np.int16)
        # S tensor
        o = slot_c - reg_off[w_c, h_c]          # region-relative offset
        o0 = o0_arr[w_c, h_c]
        p0 = 128 - (o0 % 128)                   # first subseg length cap
        rel = o - o0
        subidx = np.where(rel < p0, 0, 1 + (rel - p0) // 128)
        sub = subbase[w_c, h_c] + subidx
        row = o % 128
        dcol = dl_c - w_c * WIN
        scol = sub * WIN + dcol
        S = np.zeros((128, nsub * WIN), np.float32)
        np.add.at(S, (row, scol), 1.0)
        per_core.append(dict(idx=idx_arr, S=S))
    return meta, per_core


def _host_tensors(x, edge_index, W0, b0, Ws, bs, Wn, Wp, Wa):
    src = edge_index[0].astype(np.int64)
    dst = edge_index[1].astype(np.int64)
    deg = np.bincount(dst, minlength=N).astype(np.float64) + 1.0
    dinv = (1.0 / np.sqrt(deg)).astype(np.float32)

    meta, per_core = build_structure(src, dst)

    b_all = np.concatenate([b0[None, :], bs], axis=0).astype(np.float32)[:NL]
    Ws = Ws[:max(0, NL - 1)]
    b_exp = np.tile(b_all.reshape(1, NL * CH), (128, 1)).astype(np.float32)
    ident = np.eye(128, dtype=np.float32)

    ins = []
    for c in range(NCORES):
        lo = c * NLOC
        xs = np.ascontiguousarray(x[lo:lo + NLOC]).astype(np.float32)
        dv = np.zeros((128, NBLK), np.float32)
        dvl = dinv[lo:lo + NLOC]
        for t in range(NBLK):
            r = _blk_rows(t)
            dv[:r, t] = dvl[t * 128:t * 128 + r]
        # head masks
        gA = lo // GN
        gB = gA + 1
        pm = np.zeros((128, NBLK, 2), np.float32)
        mA = np.zeros((128, NBLK), np.float32)
        mB = np.zeros((128, NBLK), np.float32)
        node = lo + np.arange(NLOC)
        gof = node // GN
        for t in range(NBLK):
            r = _blk_rows(t)
            g_t = gof[t * 128:t * 128 + r]
            pm[:r, t, 0] = (g_t == gA)
            pm[:r, t, 1] = (g_t == gB)
            mA[:r, t] = (g_t == gA)
            mB[:r, t] = (g_t == gB)
        ohp = np.zeros((2, 16), np.float32)
        ohp[0, gA] = 1.0
        if gB < NGR:
            ohp[1, gB] = 1.0
        oh2 = np.zeros((16, 256), np.float32)
        oh2[gA, 0:128] = 1.0
        if gB < NGR:
            oh2[gB, 128:256] = 1.0
        ins.append({
            "x_sl": xs,
            "idx_in": per_core[c]["idx"],
            "s_in": per_core[c]["S"].astype(mybir.dt.np(bf16)),
            "dinv_t": dv,
            "W0_in": W0.astype(np.float32),
            "Ws_in": np.ascontiguousarray(
                Ws.transpose(1, 0, 2).reshape(CH, (NL - 1) * CH)
            ).astype(np.float32) if NL > 1 else np.zeros((CH, max(1, (NL - 1) * CH)), np.float32),
            "b_exp": b_exp,
            "ident_in": ident,
            "Wn_in": Wn.astype(np.float32),
            "Wp_in": Wp.astype(np.float32),
            "Wa1_in": Wa[:CH].astype(np.float32),
            "Wa2_in": Wa[CH:].astype(np.float32),
            "pm_in": pm.reshape(128, NBLK * 2),
            "mA_in": mA,
            "mB_in": mB,
            "ohp_in": ohp,
            "oh2_in": oh2,
        })
    return meta, ins


def build_nc(meta):
    AL = mybir.AluOpType
    AF = mybir.ActivationFunctionType
    from concourse.bass import broadcast_tensor_aps
    nc = bacc.Bacc("TRN2", num_devices=NCORES, target_bir_lowering=False,
                   debug=False, num_swdge_queues=4)

    def ts_mult(out_ap, in_ap, col_ap):
        # per-partition scale via stride-0 broadcast; avoids the DVE
        # const-pointer tensor_scalar path (~1.3us per op)
        a, b = broadcast_tensor_aps(in_ap, col_ap)
        nc.vector.tensor_tensor(out_ap, a, b, op=AL.mult)
    TOT16 = meta["TOT16"]
    NSUB = meta["NSUB"]
    supers = meta["supers"]

    # ---- dram I/O
    x_d = nc.dram_tensor("x_sl", [NLOC, CIN], f32, kind="ExternalInput")
    idx_d = nc.dram_tensor("idx_in", [128, TOT16], i16, kind="ExternalInput")
    s_d = nc.dram_tensor("s_in", [128, NSUB * WIN], bf16, kind="ExternalInput")
    dinv_d = nc.dram_tensor("dinv_t", [128, NBLK], f32, kind="ExternalInput")
    W0_d = nc.dram_tensor("W0_in", [CIN, CH], f32, kind="ExternalInput")
    Ws_d = nc.dram_tensor("Ws_in", [CH, max(1, (NL - 1) * CH)], f32,
                          kind="ExternalInput")
    b_d = nc.dram_tensor("b_exp", [128, NL * CH], f32, kind="ExternalInput")
    id_d = nc.dram_tensor("ident_in", [128, 128], f32, kind="ExternalInput")
    Wn_d = nc.dram_tensor("Wn_in", [CH, CH], f32, kind="ExternalInput")
    Wp_d = nc.dram_tensor("Wp_in", [CH, CH], f32, kind="ExternalInput")
    Wa1_d = nc.dram_tensor("Wa1_in", [CH, 1], f32, kind="ExternalInput")
    Wa2_d = nc.dram_tensor("Wa2_in", [CH, 1], f32, kind="ExternalInput")
    pm_d = nc.dram_tensor("pm_in", [128, NBLK * 2], f32, kind="ExternalInput")
    mA_d = nc.dram_tensor("mA_in", [128, NBLK], f32, kind="ExternalInput")
    mB_d = nc.dram_tensor("mB_in", [128, NBLK], f32, kind="ExternalInput")
    ohp_d = nc.dram_tensor("ohp_in", [2, 16], f32, kind="ExternalInput")
    oh2_d = nc.dram_tensor("oh2_in", [16, 256], f32, kind="ExternalInput")
    y_d = nc.dram_tensor("y", [NLOC, 1], f32, kind="ExternalOutput")

    with tile.TileContext(nc) as tc:
        import contextlib
        with contextlib.ExitStack() as ctx:
            cp = ctx.enter_context(tc.tile_pool(name="const", bufs=1))
            sa = ctx.enter_context(tc.tile_pool(name="stageA", bufs=3))
            hp = ctx.enter_context(tc.tile_pool(name="hbuf", bufs=2))
            tp = ctx.enter_context(tc.tile_pool(name="tbl", bufs=2))
            gp = ctx.enter_context(tc.tile_pool(
                name="gbuf", bufs=int(os.environ.get("KERNEL_GBUFS", "4"))))
            ep = ctx.enter_context(tc.tile_pool(name="epi", bufs=3))
            dp = ctx.enter_context(tc.tile_pool(name="dram", bufs=2,
                                                space="DRAM"))
            psA = ctx.enter_context(tc.tile_pool(name="psA", bufs=3,
                                                 space="PSUM"))
            psB = ctx.enter_context(tc.tile_pool(name="psB", bufs=5,
                                                 space="PSUM"))

            nc.gpsimd.load_library(library_config.mlp)

            def ld(tag, shape, dt_, dram):
                t = cp.tile(shape, dt_, tag=tag)
                nc.sync.dma_start(t[:], dram[:])
                return t

            idx_sb = ld("c_idx", [128, TOT16], i16, idx_d)
            s8_sb = ld("c_s8", [128, NSUB * WIN], bf16, s_d)
            dinv_sb = ld("c_dinv", [128, NBLK], f32, dinv_d)
            W0_sb = ld("c_W0", [CIN, CH], f32, W0_d)
            Ws_sb = ld("c_Ws", [CH, max(1, (NL - 1) * CH)], f32, Ws_d)
            b_sb = ld("c_b", [128, NL * CH], f32, b_d)
            id_sb = ld("c_id", [128, 128], f32, id_d)
            Wn_sb = ld("c_Wn", [CH, CH], f32, Wn_d)
            Wp_sb = ld("c_Wp", [CH, CH], f32, Wp_d)
            Wa1_sb = ld("c_Wa1", [CH, 1], f32, Wa1_d)
            Wa2_sb = ld("c_Wa2", [CH, 1], f32, Wa2_d)
            pm_sb = ld("c_pm", [128, NBLK * 2], f32, pm_d)
            mA_sb = ld("c_mA", [128, NBLK], f32, mA_d)
            mB_sb = ld("c_mB", [128, NBLK], f32, mB_d)
            ohp_sb = ld("c_ohp", [2, 16], f32, ohp_d)
            oh2_sb = ld("c_oh2", [16, 256], f32, oh2_d)

            def stage_a_block(in_ap, W_ap, Cin, tbl_t, r, t):
                p1 = psA.tile([128, 128], f32, tag="ps")
                nc.tensor.transpose(p1[:Cin, :r], in_ap, id_sb[:r, :r])
                hT = sa.tile([128, 128], f32, tag="hT")
                nc.scalar.copy(hT[:Cin, :r], p1[:Cin, :r])
                p2 = psA.tile([CH, 128], f32, tag="ps")
                nc.tensor.matmul(p2[:, :r], W_ap, hT[:Cin, :r],
                                 start=True, stop=True)
                hwT = sa.tile([CH, 128], f32, tag="hwT")
                nc.scalar.copy(hwT[:, :r], p2[:, :r])
                p3 = psA.tile([128, CH], f32, tag="ps")
                nc.tensor.transpose(p3[:r, :], hwT[:, :r], id_sb[:CH, :CH])
                ts_mult(tbl_t[:r, t, 0:CH], p3[:r, :], dinv_sb[:r, t:t + 1])

            AG_HALF = os.environ.get("KERNEL_AGHALF", "0") == "1"

            def do_allgather(tbl_t):
                bw = CH if AG_HALF else 2 * CH
                bounce = dp.tile([NLOC, bw], bf16, tag="bounce")
                nfull = (NBLK - 1) * 128
                nc.sync.dma_start(
                    bounce[0:nfull, :].rearrange("(t p) c -> p t c", p=128),
                    tbl_t[:, 0:NBLK - 1, 0:bw])
                lr = _blk_rows(NBLK - 1)
                nc.sync.dma_start(bounce[nfull:nfull + lr, :],
                                  tbl_t[0:lr, NBLK - 1, 0:bw])
                table_t = dp.tile([N, 2 * CH], bf16, tag="table",
                                  addr_space="Shared")
                nc.gpsimd.collective_compute(
                    "AllGather", AL.bypass,
                    replica_groups=[list(range(NCORES))],
                    ins=[bounce.opt()], outs=[table_t[:, 0:bw]])
                return table_t

            s1_sb = cp.tile([128, NBLK], f32, tag="s1")

            def s1_block(h_t, r, t):
                q1 = psA.tile([CH, 128], f32, tag="ps")
                nc.tensor.transpose(q1[:, :r], h_t[:r, t, :], id_sb[:r, :r])
                hT9 = sa.tile([CH, 128], f32, tag="hT9")
                nc.scalar.copy(hT9[:, :r], q1[:, :r])
                q2 = psA.tile([CH, 128], f32, tag="ps")
                nc.tensor.matmul(q2[:, :r], Wn_sb[:], hT9[:, :r], start=True,
                                 stop=True)
                rl = sa.tile([CH, 128], f32, tag="rl")
                nc.scalar.activation(rl[:, :r], q2[:, :r], AF.Relu)
                q3 = psA.tile([128, 1], f32, tag="ps")
                nc.tensor.matmul(q3[:r, :], rl[:, :r], Wa1_sb[:], start=True,
                                 stop=True, tile_position=(0, 0))
                nc.vector.tensor_copy(s1_sb[:r, t:t + 1], q3[:r, :])

            # ------- prologue: stage A for layer 0 (from x) + AllGather ----
            tbl = tp.tile([128, NBLK, 2 * CH], bf16, tag="tbl")
            for t in range(NBLK):
                r = _blk_rows(t)
                xin = sa.tile([128, CIN], f32, tag="xin")
                nc.sync.dma_start(xin[:r, :], x_d[t * 128:t * 128 + r, :])
                stage_a_block(xin[:r, :], W0_sb[:CIN, :], CIN, tbl, r, t)
            table = do_allgather(tbl)

            h_cur = None
            for layer in range(NL):
                # stage A of layer+1 is pipelined into this layer's scatter;
                # its table is all-gathered at the end of this iteration.
                if layer < NL - 1:
                    tbl_nx = tp.tile([128, NBLK, 2 * CH], bf16, tag="tbl",
                                     name=f"tblnx{layer}")
                    W_nx = Ws_sb[:CH, layer * CH:(layer + 1) * CH]
                else:
                    tbl_nx = None
                    W_nx = None
                # ------- stage B: gather + scatter matmuls -------
                h_next = hp.tile([128, NBLK, CH], f32, tag="h")
                DBG = int(os.environ.get("KERNEL_DEBUG", "0"))
                _gq = [0]
                for s, sup in enumerate(supers):
                    G = {}
                    for hh in (0, 1):
                        off, q = sup["halves"][hh]
                        g = gp.tile([128, (q + 127) // 128, 2 * CH], bf16,
                                    tag="G")
                        if DBG >= 2:
                            nc.vector.memset(g[:], 0.0)
                        else:
                            a = 0
                            while a < q:
                                nn_ = min(GCHUNK, q - a)
                                nc.gpsimd.dma_gather(
                                    g[:, a // 128:(a + nn_ + 127) // 128, :],
                                    table[hh * HALF:(hh + 1) * HALF, :],
                                    idx_sb[:, (off + a) // 16:
                                           (off + a + nn_) // 16],
                                    nn_, nn_, 2 * CH,
                                    queue_num=_gq[0] % 4)
                                _gq[0] += 1
                                a += nn_
                        G[hh] = g

                    for brec in sup["blocks"]:
                        t = brec["t"]
                        r = _blk_rows(t)
                        agg = ep.tile([128, CH], f32, tag="agg")
                        for wrec in brec["wins"]:
                            width = wrec["width"]
                            mp = wrec["mpos"]
                            pieces = (wrec["pieces"] if DBG < 1
                                      else wrec["pieces"][:1])
                            for pc in pieces:
                                st = pc["start"] if DBG < 1 else True
                                sp_ = pc["stop"] if DBG < 1 else True
                                pw = brec.setdefault("_pw", {})
                                if wrec["w"] not in pw:
                                    pwt_new = psB.tile(
                                        [128, CH], f32, tag="pt", name=f"pw{wrec['w']}")
                                    pw[wrec["w"]] = pwt_new
                                scol_abs = pc["sub"] * WIN
                                nc.tensor.matmul(
                                    pw[wrec["w"]][pc["k_lo"]:
                                                  pc["k_lo"] + width, :],
                                    s8_sb[pc["k_lo"]:pc["k_lo"] + pc["K"],
                                          scol_abs:scol_abs + width],
                                    G[pc["h"]][pc["k_lo"]:pc["k_lo"] + pc["K"],
                                               pc["gcol"], 0:CH],
                                    start=st, stop=sp_,
                                    tile_position=(pc["k_lo"], pc["k_lo"]))
                            # fold the row-group slots into agg[mp:mp+width]
                            pwt = brec["_pw"][wrec["w"]]
                            gs = wrec["groups"] if DBG < 1 else wrec["groups"][:1]
                            nc.vector.tensor_copy(
                                agg[mp:mp + width, :],
                                pwt[gs[0]:gs[0] + width, :])
                            for gk in gs[1:]:
                                nc.vector.tensor_tensor(
                                    agg[mp:mp + width, :],
                                    agg[mp:mp + width, :],
                                    pwt[gk:gk + width, :], op=AL.add)
                        brec.pop("_pw", None)
                        # epilogue: h' = relu(dinv*(agg + table_row) + b)
                        tmp = ep.tile([128, CH], f32, tag="tmp")
                        nc.vector.tensor_tensor(tmp[:r, :], agg[:r, :],
                                                tbl[:r, t, 0:CH], op=AL.add)
                        ts_mult(tmp[:r, :], tmp[:r, :], dinv_sb[:r, t:t + 1])
                        nc.vector.tensor_tensor(
                            tmp[:r, :], tmp[:r, :],
                            b_sb[:r, layer * CH:(layer + 1) * CH], op=AL.add)
                        nc.scalar.activation(h_next[:r, t, :], tmp[:r, :],
                                             AF.Relu)
                        # pipelined: next layer's table row / head s1 term
                        if tbl_nx is not None:
                            stage_a_block(h_next[:r, t, :], W_nx, CH,
                                          tbl_nx, r, t)
                        elif layer == NL - 1:
                            s1_block(h_next, r, t)
                if tbl_nx is not None:
                    tbl = tbl_nx
                    table = do_allgather(tbl)
                h_cur = h_next

            # ---------------- head ----------------
            # pool partials [2, 64] via mask matmuls
            pp = psB.tile([2, CH], f32, tag="pt")
            for t in range(NBLK):
                r = _blk_rows(t)
                nc.tensor.matmul(pp[:, :], pm_sb[:r, 2 * t:2 * t + 2],
                                 h_cur[:r, t, :],
                                 start=(t == 0), stop=(t == NBLK - 1),
                                 tile_position=(0, 0))
            pp_sb = sa.tile([2, CH], f32, tag="pp")
            nc.vector.tensor_copy(pp_sb[:], pp[:])
            pg = psB.tile([16, CH], f32, tag="pt")
            nc.tensor.matmul(pg[:], ohp_sb[:], pp_sb[:], start=True, stop=True,
                             tile_position=(0, 0))
            pg_sb = sa.tile([16, CH], f32, tag="pg")
            nc.vector.tensor_copy(pg_sb[:], pg[:])
            b2 = dp.tile([16, CH], f32, tag="b2")
            nc.sync.dma_start(b2[:], pg_sb[:])
            ar = dp.tile([16, CH], f32, tag="ar", addr_space="Shared")
            nc.gpsimd.collective_compute(
                "AllReduce", AL.add, replica_groups=[list(range(NCORES))],
                ins=[b2.opt()], outs=[ar.opt()])
            pool_sb = sa.tile([16, CH], f32, tag="pool")
            nc.sync.dma_start(pool_sb[:], ar[:])
            # poolT [64, 16]
            pT = psA.tile([CH, 16], f32, tag="ps")
            nc.tensor.transpose(pT[:], pool_sb[:], id_sb[:16, :16])
            pT_sb = sa.tile([CH, 16], f32, tag="pT")
            nc.vector.tensor_copy(pT_sb[:], pT[:])
            pWn = psA.tile([CH, 16], f32, tag="ps")
            nc.tensor.matmul(pWn[:], Wn_sb[:], pT_sb[:], start=True, stop=True)
            pWn_sb = sa.tile([CH, 16], f32, tag="pWn")
            nc.vector.tensor_copy(pWn_sb[:], pWn[:])
            rep = psA.tile([CH, 16], f32, tag="ps")
            nc.tensor.matmul(rep[:], Wp_sb[:], pWn_sb[:], start=True, stop=True)
            repr_sb = sa.tile([CH, 16], f32, tag="repr")
            nc.scalar.activation(repr_sb[:], rep[:], AF.Relu)
            s2 = psA.tile([16, 1], f32, tag="ps")
            nc.tensor.matmul(s2[:], repr_sb[:], Wa2_sb[:], start=True,
                             stop=True, tile_position=(0, 0))
            s2_sb = sa.tile([16, 1], f32, tag="s2")
            nc.vector.tensor_copy(s2_sb[:], s2[:])
            s2A = psA.tile([128, 1], f32, tag="ps")
            nc.tensor.matmul(s2A[:], oh2_sb[:, 0:128], s2_sb[:], start=True,
                             stop=True, tile_position=(0, 0))
            s2A_sb = sa.tile([128, 1], f32, tag="s2A")
            nc.vector.tensor_copy(s2A_sb[:], s2A[:])
            s2B = psA.tile([128, 1], f32, tag="ps")
            nc.tensor.matmul(s2B[:], oh2_sb[:, 128:256], s2_sb[:], start=True,
                             stop=True, tile_position=(0, 0))
            s2B_sb = sa.tile([128, 1], f32, tag="s2B")
            nc.vector.tensor_copy(s2B_sb[:], s2B[:])
            # s1 per tile was computed inline during the last layer's scatter
            # s2e + tanh
            e1 = sa.tile([128, NBLK], f32, tag="e1")
            ts_mult(e1[:], mA_sb[:], s2A_sb[:, 0:1])
            e2 = sa.tile([128, NBLK], f32, tag="e2")
            ts_mult(e2[:], mB_sb[:], s2B_sb[:, 0:1])
            nc.vector.tensor_tensor(e1[:], e1[:], e2[:], op=AL.add)
            nc.vector.tensor_tensor(e1[:], e1[:], s1_sb[:], op=AL.add)
            yo = cp.tile([128, NBLK, 1], f32, tag="yo")
            _fn = (AF.Copy if os.environ.get("KERNEL_NOTANH", "0") == "1"
                   else AF.Tanh)
            nc.scalar.activation(yo[:, :, 0], e1[:], _fn)
            nfull = (NBLK - 1) * 128
            nc.sync.dma_start(
                y_d[0:nfull, :].rearrange("(t p) c -> p t c", p=128),
                yo[:, 0:NBLK - 1, :])
            lr = _blk_rows(NBLK - 1)
            nc.sync.dma_start(y_d[nfull:nfull + lr, :], yo[0:lr, NBLK - 1, :])
    nc.compile()
    return nc


def kernel(x, edge_index, W0, b0, Ws, bs, Wn, Wp, Wa, n):
    global LAST_EXEC_NS, LAST_TRACE
    assert int(n) == GN
    x = np.asarray(x, np.float32)
    edge_index = np.asarray(edge_index)
    meta, ins = _host_tensors(np.asarray(x), np.asarray(edge_index),
                              np.asarray(W0), np.asarray(b0), np.asarray(Ws),
                              np.asarray(bs), np.asarray(Wn), np.asarray(Wp),
                              np.asarray(Wa))
    nc = build_nc(meta)
    trace = os.environ.get("KERNEL_TRACE", "0") == "1"
    if trace:
        import types
        try:
            import antenv.axon_hooks  # noqa: F401
        except ImportError:
            from trn_agent_boot.trn_boot import _ntff_profile_via_ctypes
            _hook = _ntff_profile_via_ctypes('/opt/axon/libaxon_pjrt.so')
            mod = types.ModuleType("antenv.axon_hooks")
            mod.get_axon_ntff_profile_hook = lambda: _hook
            sys.modules['antenv.axon_hooks'] = mod
    res = run_bass_kernel_spmd(
        nc, ins, core_ids=list(range(NCORES)), trace=trace,
        trace_cores=list(range(NCORES)) if trace else None)
    LAST_EXEC_NS = res.exec_time_ns
    LAST_TRACE = (res.instructions_and_trace[1]
                  if res.instructions_and_trace else None)
    out = np.empty((N, 1), np.float32)
    for c in range(NCORES):
        out[c * NLOC:(c + 1) * NLOC] = res.results[c]["y"]
    return out

